# revision 15
# baseline (speedup 1.0000x reference)
"""Bass/Trainium2 kernel for nn_BottomUpIntegrator (segment_reduce).

Strategy (data-parallel over cells, 8 cores):
  - Host packs x~ = [cell_state; arch_state; ones] transposed to [37, N] so the
    per-cell MLP contracts over features on the PE partition dim with cells
    streaming on the free dim (memory-regime friendly: every row is contiguous).
  - Device (per core, L = N/8 cells): for each 4096-cell supertile
      mm1 (2-way col-group packed): h[128, 512] = [W1;b1]^T @ x~  (two 512-cell
        chunks -> psum rows 0:64 and 64:128)
      relu: ACT psum->sbuf [128, 512]
      mm2 (4 col-groups): s[row 32*(c//2), 512*(c%2)] = W2^T @ relu_h
      DVE copy s rows {0,32,64,96} -> sbuf, DMA rows out to DRAM in cell order.
  - Host: sigmoid/clip/w chain, segment sums (bincount), tiny [C]-level stage.
"""

import sys

sys.path.insert(0, "/opt/trn_rl_repo")

import numpy as np

N_CORES = 8
N_TOTAL = 1048576
L = N_TOTAL // N_CORES          # 131072 cells per core
SUP = 4096                      # cells per supertile
N_SUP = L // SUP                # 32 supertiles
CHUNK = 512                     # cells per matmul chunk
PAIRS = [(0, 2), (1, 3), (4, 6), (5, 7)]  # (A-side, B-side) chunk ids

_cache = {}


def _build_kernel():
    if "nc" in _cache:
        return _cache["nc"]
    import concourse.bacc as bacc
    import concourse.mybir as mybir
    from concourse.tile import TileContext

    fp32 = mybir.dt.float32
    nc = bacc.Bacc(None, target_bir_lowering=False)

    X = nc.dram_tensor("X", [37, L], fp32, kind="ExternalInput")
    Wmlp = nc.dram_tensor("Wmlp", [37, 128], fp32, kind="ExternalInput")
    W2s = nc.dram_tensor("W2s", [128, 1], fp32, kind="ExternalInput")
    s_out = nc.dram_tensor("s_out", [N_SUP * 3, 1536], fp32, kind="ExternalOutput")

    with TileContext(nc) as tc:
        with (
            tc.tile_pool(name="const", bufs=1) as cpool,
            tc.tile_pool(name="xp", bufs=2) as xpool,
            tc.tile_pool(name="rp", bufs=3) as rpool,
            tc.tile_pool(name="sp", bufs=2) as sspool,
            tc.tile_pool(name="hps", bufs=2, space="PSUM") as hpsum,
            tc.tile_pool(name="sps", bufs=2, space="PSUM") as spsum,
        ):
            w1_t = cpool.tile([37, 128], fp32)
            w2_t = cpool.tile([128, 1], fp32)
            nc.sync.dma_start(w1_t[:], Wmlp[:])
            nc.sync.dma_start(w2_t[:], W2s[:])
            # PE matmuls accept only ONE sync wait in HW; absorb each DMA
            # completion onto its own dummy matmul so real matmuls need <=1.
            bias_t = cpool.tile([128, 1], fp32)
            dmy_sb = cpool.tile([1, 2], fp32)
            nc.vector.memset(bias_t[:], 0.0)
            d_ps = hpsum.tile([128, CHUNK], fp32, tag="h_ps")
            nc.tensor.matmul(d_ps[0:1, 0:1], w1_t[0:1, 0:1], w1_t[0:1, 0:1],
                             start=True, stop=True)
            nc.tensor.matmul(d_ps[0:1, 1:2], w2_t[0:1, 0:1], w2_t[0:1, 0:1],
                             start=True, stop=True)
            # ACT absorbers: bias tile (DVE memset) and weight DMAs
            nc.scalar.activation(dmy_sb[0:1, 0:1], bias_t[0:1, 0:1],
                                 mybir.ActivationFunctionType.Relu,
                                 bias=bias_t[0:1, 0:1])
            nc.scalar.activation(dmy_sb[0:1, 1:2], w1_t[0:1, 0:1],
                                 mybir.ActivationFunctionType.Relu,
                                 bias=bias_t[0:1, 0:1])

            for sup in range(N_SUP):
                x_t = xpool.tile([37, SUP], fp32)
                nc.sync.dma_start(x_t[:], X[:, sup * SUP:(sup + 1) * SUP])
                s_ps = spsum.tile([128, 1536], fp32)
                s_sb = sspool.tile([65, 1536], fp32)
                # wait-absorbers: s_ps release (DVE) then x_t DMA
                nc.tensor.matmul(s_ps[64:65, 1535:1536], w1_t[0:1, 0:1],
                                 w1_t[0:1, 0:1], start=True, stop=True)
                nc.tensor.matmul(s_ps[64:65, 1534:1535], x_t[0:1, 0:1],
                                 x_t[0:1, 0:1], start=True, stop=True)
                for (ca, cb) in PAIRS:
                    h_ps = hpsum.tile([128, CHUNK], fp32)
                    nc.tensor.matmul(
                        h_ps[0:64, :], w1_t[:, 0:64],
                        x_t[:, ca * CHUNK:(ca + 1) * CHUNK],
                        start=True, stop=True,
                    )
                    nc.tensor.matmul(
                        h_ps[64:128, :], w1_t[:, 64:128],
                        x_t[:, cb * CHUNK:(cb + 1) * CHUNK],
                        start=True, stop=True,
                    )
                    r_t = rpool.tile([128, CHUNK], fp32)
                    nc.scalar.activation(
                        r_t[:], h_ps[:],
                        mybir.ActivationFunctionType.Relu,
                        bias=bias_t[:],
                    )
                    for c, rows in ((ca, slice(0, 64)), (cb, slice(64, 128))):
                        row = 32 * (c // 3)
                        col = (c % 3) * CHUNK
                        nc.tensor.matmul(
                            s_ps[row:row + 1, col:col + CHUNK],
                            w2_t[rows, 0:1], r_t[rows, :],
                            start=True, stop=True,
                        )
                # absorb s_sb release (DMA-out sem) on a tiny DVE op first
                nc.vector.memset(s_sb[64:65, 1535:1536], 0.0)
                nc.vector.tensor_copy(s_sb[:], s_ps[0:65, :])
                for r in range(3):
                    nc.sync.dma_start(
                        s_out[sup * 3 + r:sup * 3 + r + 1, :],
                        s_sb[32 * r:32 * r + 1, :],
                    )
    nc.finalize()
    _cache["nc"] = nc
    return nc


def kernel(cell_state, arch_state, energy, phi_local,
           W1, b1, W2, b2, Wc1, bc1, Wc2, bc2,
           segment_ids, n_clusters):
    from concourse.bass_utils import run_bass_kernel_spmd

    cell_state = np.asarray(cell_state, np.float32)
    arch_state = np.asarray(arch_state, np.float32)
    energy = np.asarray(energy, np.float32)
    phi_local = np.asarray(phi_local, np.float32)
    W1 = np.asarray(W1, np.float32)
    b1 = np.asarray(b1, np.float32)
    W2 = np.asarray(W2, np.float32)
    b2 = np.asarray(b2, np.float32)
    Wc1 = np.asarray(Wc1, np.float32)
    bc1 = np.asarray(bc1, np.float32)
    Wc2 = np.asarray(Wc2, np.float32)
    bc2 = np.asarray(bc2, np.float32)
    seg = np.asarray(segment_ids, np.int32)
    C = int(np.asarray(n_clusters))
    n = cell_state.shape[0]

    # ---- host packing ----
    Xfull = np.empty((37, n), np.float32)
    Xfull[0:32] = cell_state.T
    Xfull[32:36] = arch_state.T
    Xfull[36] = 1.0
    W1b = np.concatenate([W1, b1[None, :]], axis=0)          # [37, 64]
    Wmlp = np.ascontiguousarray(np.concatenate([W1b, W1b], axis=1))  # [37,128]
    W2s = np.ascontiguousarray(np.concatenate([W2, W2], axis=0))     # [128,1]

    nc = _build_kernel()
    in_maps = []
    for i in range(N_CORES):
        in_maps.append({
            "X": np.ascontiguousarray(Xfull[:, i * L:(i + 1) * L]),
            "Wmlp": Wmlp,
            "W2s": W2s,
        })
    res = run_bass_kernel_spmd(nc, in_maps, core_ids=list(range(N_CORES)))
    # s_out rows: [sup, r(3)] x [slot(3) * 512]; chunk = r*3+slot, first 8 valid
    s_parts = []
    for r in res.results:
        arr = r["s_out"].reshape(N_SUP, 9, CHUNK)[:, 0:8, :]
        s_parts.append(arr.reshape(-1))
    s = np.concatenate(s_parts)

    # ---- host epilogue (cheap [C]-level stage) ----
    base = 1.0 / (1.0 + np.exp(-(s.astype(np.float64) + float(b2[0]))))
    e = energy.astype(np.float64)
    p = phi_local.astype(np.float64)
    importance = np.clip(base * e * p, 0.01, 1.0)
    w = importance * e * p

    def seg_sum(v):
        return np.bincount(seg, weights=v, minlength=C)

    counts = seg_sum(np.ones_like(w))
    wsum = seg_sum(w)
    safe_cnt = np.maximum(counts, 1.0)
    a = arch_state.astype(np.float64)
    wa = np.stack([seg_sum(w * a[:, k]) for k in range(a.shape[1])], axis=1)
    a1 = np.stack([seg_sum(a[:, k]) for k in range(a.shape[1])], axis=1)
    a2 = np.stack([seg_sum(a[:, k] ** 2) for k in range(a.shape[1])], axis=1)

    aggregate = np.where(
        (wsum > 0)[:, None],
        wa / np.maximum(wsum, 1e-30)[:, None],
        a1 / safe_cnt[:, None],
    )
    mean = a1 / safe_cnt[:, None]
    sqdev = a2 - 2.0 * mean * a1 + counts[:, None] * mean ** 2
    var = np.where(
        counts >= 2.0,
        sqdev.mean(-1) / np.maximum(counts - 1.0, 1.0),
        0.0,
    )
    phi_cluster = 1.0 - np.minimum(1.0, var * 2.0)
    coherence = 1.0 - var

    feats = np.concatenate(
        [aggregate, phi_cluster[:, None], coherence[:, None],
         np.minimum(1.0, counts / 20.0)[:, None]], axis=1)
    hc = np.maximum(feats @ Wc1 + bc1, 0.0)
    basec = 1.0 / (1.0 + np.exp(-(hc @ Wc2 + bc2)))[:, 0]
    impc = np.clip(basec * phi_cluster, 0.01, 1.0)
    valid = counts > 0
    wc = np.where(valid, impc * counts, 0.0)
    wc_sum = wc.sum()
    nvalid = max(float(valid.sum()), 1.0)
    norm_wc = np.where(wc_sum > 0, wc / max(wc_sum, 1e-30),
                       valid.astype(np.float64) / nvalid)
    global_archetype = (norm_wc[:, None] * aggregate).sum(0)

    return (aggregate.astype(np.float32), phi_cluster.astype(np.float32),
            coherence.astype(np.float32), global_archetype.astype(np.float32))
